# revision 9
# baseline (speedup 1.0000x reference)
"""Trainium2 Bass kernel for nn_CoAttentionLayer2 (dense_transformer).

Sharding: pure data parallel — batch B=8 mapped 1:1 onto 8 NeuronCores.
Each core runs the full co-attention layer for one batch element; no
collectives. Weights are replicated (cast to fp16 on host).

v4 structure (baseline was 191965ns):
  * x prefetched to SBUF as fp16 (host cast) in 2 big DMAs per tensor —
    per-DMA completion latency (~2.5us) made 16 per-tile loads serialize.
  * xhat -> xhatT via XBAR dma_start_transpose (2-byte dtype) on the
    otherwise-idle sync ring: no PE transposes, no PSUM->SBUF copyback.
  * LN affine on ScalarE (per-partition scale/bias APs); stats chain on
    DVE — the two engines split the LN cost evenly.
  * All PE operands fp16: less PE power (HAM throttles fp32 kernels to
    K=4/8), fast-weight-load enabled, 1 cycle/row.
  * Per-head-pair attention: dots for both heads concurrently (disjoint
    PE row groups via tile_position), exp on ScalarE, attn@v with an
    augmented ones-column producing softmax row-sums for free.
  * Injection queue: every attention iteration can carry one extra PE
    work item (trailing V tiles, next pairs' K/Q chunks, previous pair's
    O-projection partials), so the PE never drains between phases.
  * Normalize split per query half; O projection accumulated in SBUF,
    only the last pair's partial remains after attention.
"""

import numpy as np

import concourse.bass as bass
import concourse.mybir as mybir
import concourse.tile as tile
from concourse import bacc
from concourse.bass_utils import run_bass_kernel_spmd

P = 128
B = 8
N = 1024  # tokens (queries == keys)
D = 512  # model dim
HEADS = 8
DH = 64
INNER = 512
SCALE = DH**-0.5
EPS = 1e-5
F32 = mybir.dt.float32
F16 = mybir.dt.float16

KO = D // P  # 4 contraction tiles
JT = INNER // P  # 4 output-feature tiles (== head pairs)
TT = N // P  # 8 token tiles
IC = 2  # query chunks of 512
NQC = N // IC  # 512
LAG = 6  # attn@v trails dots/exp by this many (kt, ic) steps


def _build_nc():
    nc = bacc.Bacc(
        "TRN2",
        target_bir_lowering=False,
        debug=False,
        num_devices=B,
    )

    xq_d = nc.declare_dram_parameter("xq", [N, D], F16, isOutput=False)
    xkv_d = nc.declare_dram_parameter("xkv", [N, D], F16, isOutput=False)
    wq_d = nc.declare_dram_parameter("wq", [D, INNER], F16, isOutput=False)
    wk_d = nc.declare_dram_parameter("wk", [D, INNER], F16, isOutput=False)
    wv_d = nc.declare_dram_parameter("wv", [D, INNER], F16, isOutput=False)
    wo_d = nc.declare_dram_parameter("wo", [INNER, D], F16, isOutput=False)
    bq_d = nc.declare_dram_parameter("bq", [INNER], F32, isOutput=False)
    bk_d = nc.declare_dram_parameter("bk", [INNER], F32, isOutput=False)
    bv_d = nc.declare_dram_parameter("bv", [INNER], F32, isOutput=False)
    out_d = nc.declare_dram_parameter("out", [N, D], F32, isOutput=True)

    with tile.TileContext(nc) as tc:
        with (
            tc.tile_pool(name="singles", bufs=1) as singles,
            tc.tile_pool(name="big", bufs=1) as big,
            tc.tile_pool(name="work", bufs=3) as work,
            tc.tile_pool(name="ps", bufs=2, space="PSUM") as ps,
        ):
            # ---- inputs / weights / constants ----
            xin_kv = big.tile([P, TT, D], F16)
            xin_q = big.tile([P, TT, D], F16)
            H = TT // 2
            for h in range(2):
                nc.sync.dma_start(
                    out=xin_kv[:, h * H : (h + 1) * H, :],
                    in_=xkv_d[h * H * P : (h + 1) * H * P, :].rearrange(
                        "(tt p) j -> p tt j", p=P
                    ),
                )
            for h in range(2):
                nc.scalar.dma_start(
                    out=xin_q[:, h * H : (h + 1) * H, :],
                    in_=xq_d[h * H * P : (h + 1) * H * P, :].rearrange(
                        "(tt p) j -> p tt j", p=P
                    ),
                )

            wq_sb = singles.tile([P, KO, INNER], F16)
            wk_sb = singles.tile([P, KO, INNER], F16)
            wv_sb = singles.tile([P, KO, INNER], F16)
            wo_sb = singles.tile([P, KO, D], F16)
            bq_sb = singles.tile([P, JT], F32)
            bk_sb = singles.tile([P, JT], F32)
            bvB = singles.tile([P, INNER], F32)
            # weight rows laid out to match the xbar transpose ordering:
            # feature j of xhat lands at xhatT[p=j%128, ko=j//128]
            nc.gpsimd.dma_start(out=wv_sb[:], in_=wv_d.rearrange("(ko p) j -> p ko j", p=P))
            nc.gpsimd.dma_start(out=wk_sb[:], in_=wk_d.rearrange("(ko p) j -> p ko j", p=P))
            nc.gpsimd.dma_start(out=bq_sb[:], in_=bq_d.rearrange("(t p) -> p t", p=P))
            nc.gpsimd.dma_start(out=bk_sb[:], in_=bk_d.rearrange("(t p) -> p t", p=P))
            bv_ap = bv_d.ap()
            bv_bcast = bass.AP(tensor=bv_ap.tensor, offset=bv_ap.offset, ap=[[0, P], [1, INNER]])
            nc.gpsimd.dma_start(out=bvB[:], in_=bv_bcast)
            nc.gpsimd.dma_start(out=wq_sb[:], in_=wq_d.rearrange("(ko p) j -> p ko j", p=P))
            nc.gpsimd.dma_start(out=wo_sb[:], in_=wo_d.rearrange("(co p) j -> p co j", p=P))

            eps_sb = singles.tile([P, 1], F32)
            nc.vector.memset(eps_sb, EPS)

            # ---- persistent activations ----
            xhatT_q = big.tile([P, KO, N], F16)  # [d%128, d//128, token]
            xhatT_kv = big.tile([P, KO, N], F16)
            QT = big.tile([P, JT, N], F16)  # [j%128, j//128, token]
            KT = big.tile([P, JT, N], F16)
            Vg = big.tile([P, TT, HEADS, DH + 1], F16)  # [key%128, keytile, h, dh|1]
            outT = big.tile([P, KO, N], F16)  # [c%128, c//128(=pair), token]
            oacc = big.tile([P, TT, D], F32)  # O-proj partial accumulator

            ones_sb = singles.tile([P, 1], F32)
            nc.vector.memset(ones_sb, 1.0)
            nc.vector.tensor_copy(
                out=Vg[:, :, :, DH : DH + 1],
                in_=ones_sb[:, None, None, :].to_broadcast((P, TT, HEADS, 1)),
            )

            # ---- stage emitters ----
            def ln_transpose(xin, xhatT, tt):
                """LayerNorm one token tile (DVE stats + ScalarE affine), then
                XBAR-transpose into xhatT via the sync HWDGE ring."""
                xt = xin[:, tt, :]
                stats = work.tile([P, 6], F32, tag="ln_stats")
                nc.vector.bn_stats(out=stats[:], in_=xt)
                mv = work.tile([P, 2], F32, tag="ln_mv")
                nc.vector.bn_aggr(out=mv[:], in_=stats[:])
                std = work.tile([P, 1], F32, tag="ln_std")
                nc.scalar.activation(
                    out=std[:],
                    in_=mv[:, 1:2],
                    func=mybir.ActivationFunctionType.Sqrt,
                    bias=eps_sb[:],
                    scale=1.0,
                )
                rstd = work.tile([P, 1], F32, tag="ln_rstd")
                nc.vector.reciprocal(out=rstd[:], in_=std[:])
                # nmr = -mu * rstd (per-partition bias for the ScalarE affine)
                nmr = work.tile([P, 1], F32, tag="ln_nmr")
                nc.vector.tensor_scalar(
                    out=nmr[:],
                    in0=mv[:, 0:1],
                    scalar1=rstd[:, 0:1],
                    scalar2=-1.0,
                    op0=mybir.AluOpType.mult,
                    op1=mybir.AluOpType.mult,
                )
                xhat = work.tile([P, D], F16, tag="xhat", bufs=4)
                nc.scalar.activation(
                    out=xhat[:],
                    in_=xt,
                    func=mybir.ActivationFunctionType.Identity,
                    bias=nmr[:],
                    scale=rstd[:],
                )
                nc.sync.dma_start_transpose(
                    out=xhatT[:, :, tt * P : (tt + 1) * P], in_=xhat[:]
                )

            def v_proj(tt):
                """V projection (token-major) into the augmented V tile."""
                pm = ps.tile([P, INNER], F32, tag="big")
                for ko in range(KO):
                    nc.tensor.matmul(
                        pm[:],
                        xhatT_kv[:, ko, tt * P : (tt + 1) * P],
                        wv_sb[:, ko, :],
                        start=(ko == 0),
                        stop=(ko == KO - 1),
                    )
                nc.vector.tensor_tensor(
                    out=Vg[:, tt, :, 0:DH],
                    in0=pm[:].rearrange("p (h d) -> p h d", d=DH),
                    in1=bvB.rearrange("p (h d) -> p h d", d=DH),
                    op=mybir.AluOpType.add,
                )

            def kq_proj(w_sb, b_sb, src, dstT, jt, ic):
                """K^T or Q^T projection for feature tile jt, chunk ic."""
                pm = ps.tile([P, NQC], F32, tag="big")
                for ko in range(KO):
                    nc.tensor.matmul(
                        pm[:],
                        w_sb[:, ko, jt * P : (jt + 1) * P],
                        src[:, ko, ic * NQC : (ic + 1) * NQC],
                        start=(ko == 0),
                        stop=(ko == KO - 1),
                    )
                nc.vector.tensor_scalar(
                    out=dstT[:, jt, ic * NQC : (ic + 1) * NQC],
                    in0=pm[:],
                    scalar1=b_sb[:, jt : jt + 1],
                    scalar2=None,
                    op0=mybir.AluOpType.add,
                )

            def k_proj(jt, ic):
                kq_proj(wk_sb, bk_sb, xhatT_kv, KT, jt, ic)

            def q_proj(jt, ic):
                kq_proj(wq_sb, bq_sb, xhatT_q, QT, jt, ic)

            def attention_pair(hq, queue=()):
                """Heads 2*hq and 2*hq+1 together: their dots matmuls use
                disjoint PE row groups (K=64 at base partitions 0 and 64) and
                run concurrently into different PSUM banks. One thunk from
                `queue` is emitted per (kt, ic) iteration so the PE always has
                independent fill work."""
                h0, h1 = 2 * hq, 2 * hq + 1
                po0 = ps.tile([DH + 1, N], F32, tag="attnv", name="po0")
                po1 = ps.tile([DH + 1, N], F32, tag="attnv", name="po1")
                queue = list(queue)
                exs = []
                for kt in range(TT):
                    for ic in range(IC):
                        # pd holds head-even in the first bank, head-odd in
                        # the second; the two matmuls run concurrently
                        pd = ps.tile([P, N], F32, tag="big", name="pd")
                        for hh in range(2):
                            nc.tensor.matmul(
                                pd[:, hh * NQC : (hh + 1) * NQC],
                                KT[hh * DH : (hh + 1) * DH, hq, kt * P : (kt + 1) * P],
                                QT[hh * DH : (hh + 1) * DH, hq, ic * NQC : (ic + 1) * NQC],
                                start=True,
                                stop=True,
                                tile_position=(hh * DH, 0),
                            )
                        ex = work.tile([P, N], F16, tag="expT", bufs=LAG + 2)
                        nc.scalar.activation(
                            out=ex[:],
                            in_=pd[:],
                            func=mybir.ActivationFunctionType.Exp,
                            scale=SCALE,
                        )
                        exs.append(ex)
                        i = len(exs) - 1
                        if queue:
                            thunk = queue.pop(0)
                            if thunk is not None:
                                thunk()
                        if i >= LAG:
                            _attnv_pair(po0, po1, h0, h1, i - LAG, exs[i - LAG])
                for thunk in queue:
                    if thunk is not None:
                        thunk()
                for i in range(TT * IC - LAG, TT * IC):
                    _attnv_pair(po0, po1, h0, h1, i, exs[i])
                # normalize per query half so dependents can start after the
                # (kt=7, ic) flush instead of the full pair
                for ic in range(IC):
                    _normalize(po0, h0, ic)
                    _normalize(po1, h1, ic)

            def _attnv_pair(po0, po1, h0, h1, i, ex):
                # ex holds [head0 chunk ic | head1 chunk ic] for key tile kt
                kt, ic = divmod(i, IC)
                for po, h, hh in ((po0, h0, 0), (po1, h1, 1)):
                    nc.tensor.matmul(
                        po[:, ic * NQC : (ic + 1) * NQC],
                        Vg[:, kt, h, :],
                        ex[:, hh * NQC : (hh + 1) * NQC],
                        start=(kt == 0),
                        stop=(kt == TT - 1),
                    )

            def _normalize(po, h, ic):
                # out^T = po[0:64] * (1/rowsum) broadcast over partitions
                hb = (h % 2) * DH
                hq = h // 2
                sl = slice(ic * NQC, (ic + 1) * NQC)
                rtmp = work.tile([1, 2 * NQC], F32, tag="rectmp")
                rs, rec = rtmp[:, 0:NQC], rtmp[:, NQC : 2 * NQC]
                nc.vector.tensor_copy(out=rs, in_=po[DH : DH + 1, sl])
                nc.vector.reciprocal_approx_fast(out=rec, in_=rs)
                recB = work.tile([DH, NQC], F32, tag="recB")
                nc.gpsimd.partition_broadcast(recB[:], rec[:])
                nc.vector.tensor_tensor(
                    out=outT[hb : hb + DH, hq, sl],
                    in0=po[0:DH, sl],
                    in1=recB[:],
                    op=mybir.AluOpType.mult,
                )

            def o_part(q, tt):
                """O-projection contribution of head pair q (contraction tile
                co == q) for token tile tt, accumulated into oacc; the last
                pair finishes and DMAs out."""
                pm = ps.tile([P, D], F32, tag="big")
                nc.tensor.matmul(
                    pm[:],
                    outT[:, q, tt * P : (tt + 1) * P],
                    wo_sb[:, q, :],
                    start=True,
                    stop=True,
                )
                if q == 0:
                    nc.vector.tensor_copy(out=oacc[:, tt, :], in_=pm[:])
                elif q < JT - 1:
                    nc.vector.tensor_tensor(
                        out=oacc[:, tt, :], in0=oacc[:, tt, :], in1=pm[:],
                        op=mybir.AluOpType.add,
                    )
                else:
                    ot = work.tile([P, D], F32, tag="out")
                    nc.vector.tensor_tensor(
                        out=ot[:], in0=oacc[:, tt, :], in1=pm[:],
                        op=mybir.AluOpType.add,
                    )
                    nc.sync.dma_start(out=out_d[tt * P : (tt + 1) * P, :], in_=ot[:])

            # ---- emission: minimal front (LN + first pair's K/Q + V0), then
            # everything else rides the attention injection queues ----
            for tt in range(TT):
                ln_transpose(xin_kv, xhatT_kv, tt)
            for tt in range(TT):
                ln_transpose(xin_q, xhatT_q, tt)
            v_proj(0)
            k_proj(0, 0)
            k_proj(0, 1)
            q_proj(0, 0)
            q_proj(0, 1)

            def kq(jt):
                return [
                    lambda: k_proj(jt, 0),
                    lambda: k_proj(jt, 1),
                    lambda: q_proj(jt, 0),
                    lambda: q_proj(jt, 1),
                ]

            def oparts(q):
                return [lambda tt=tt: o_part(q, tt) for tt in range(TT)]

            attention_pair(0, queue=[lambda tt=tt: v_proj(tt) for tt in range(1, TT)] + kq(1))
            attention_pair(1, queue=kq(2) + oparts(0))
            attention_pair(2, queue=kq(3) + oparts(1))
            attention_pair(3, queue=[None] * 4 + oparts(2))
            for tt in range(TT):
                o_part(3, tt)

    nc.compile()
    return nc


_NC_CACHE = {}


def _get_nc():
    if "nc" not in _NC_CACHE:
        _NC_CACHE["nc"] = _build_nc()
    return _NC_CACHE["nc"]


def _prep_in_maps(query, keyvalue, Wq, Wkv, Wo, gamma, beta):
    query = np.asarray(query, dtype=np.float32)
    keyvalue = np.asarray(keyvalue, dtype=np.float32)
    Wq = np.asarray(Wq, dtype=np.float32)
    Wkv = np.asarray(Wkv, dtype=np.float32)
    Wo = np.asarray(Wo, dtype=np.float32)
    gamma = np.asarray(gamma, dtype=np.float32)
    beta = np.asarray(beta, dtype=np.float32)

    # fold LN affine into the projections: (xhat*g + b) @ W = xhat @ (g[:,None]*W) + b @ W
    wq_eff = np.ascontiguousarray((gamma[:, None] * Wq).astype(np.float16))
    wkv_eff = gamma[:, None] * Wkv
    bq = np.ascontiguousarray(beta @ Wq)
    bkv = beta @ Wkv
    wk_eff = np.ascontiguousarray(wkv_eff[:, :INNER].astype(np.float16))
    wv_eff = np.ascontiguousarray(wkv_eff[:, INNER:].astype(np.float16))
    bk = np.ascontiguousarray(bkv[:INNER])
    bv = np.ascontiguousarray(bkv[INNER:])
    wo_eff = np.ascontiguousarray(Wo.astype(np.float16))

    return [
        dict(
            xq=np.ascontiguousarray(query[b].astype(np.float16)),
            xkv=np.ascontiguousarray(keyvalue[b].astype(np.float16)),
            wq=wq_eff,
            wk=wk_eff,
            wv=wv_eff,
            wo=wo_eff,
            bq=bq,
            bk=bk,
            bv=bv,
        )
        for b in range(B)
    ]


def run_sharded(inputs, **spmd_kwargs):
    """Run the SPMD kernel; returns (stacked output [B, N, D], BassKernelResults)."""
    nc = _get_nc()
    in_maps = _prep_in_maps(**inputs)
    r = run_bass_kernel_spmd(nc, in_maps, core_ids=list(range(B)), **spmd_kwargs)
    out = np.stack([r.results[b]["out"] for b in range(B)], axis=0)
    return out, r


def kernel(query, keyvalue, Wq, Wkv, Wo, gamma, beta):
    out, _ = run_sharded(
        dict(query=query, keyvalue=keyvalue, Wq=Wq, Wkv=Wkv, Wo=Wo, gamma=gamma, beta=beta)
    )
    return out


# revision 13
# speedup vs baseline: 1.0238x; 1.0238x over previous
"""Trainium2 Bass kernel for nn_CoAttentionLayer2 (dense_transformer).

Sharding: pure data parallel — batch B=8 mapped 1:1 onto 8 NeuronCores.
Each core runs the full co-attention layer for one batch element; no
collectives. Weights are replicated (cast to fp16 on host).

v5 structure (baseline was 191965ns):
  * x prefetched to SBUF as fp16 (host cast) in 2 big DMAs per tensor —
    per-DMA completion latency (~2.5us) made 16 per-tile loads serialize.
  * kv path: xhat -> xhatT via XBAR dma_start_transpose on the sync
    HWDGE ring. q path: PE transposes (PE is idle during the front) with
    gpsimd PSUM->SBUF copyback — one ring cannot carry all 16 transposes
    in time.
  * LN: DVE stats chain + ScalarE affine (per-partition scale/bias APs,
    -mu*rstd built on gpsimd); kv/q tiles interleaved 4-and-4 so the DVE
    stream never stalls on a half-tensor DMA arrival.
  * All PE operands fp16: less PE power (HAM throttles fp32 kernels to
    K=4/8), fast-weight-load enabled, 1 cycle/row.
  * Per-head-pair attention: dots for both heads concurrently (disjoint
    PE row groups via tile_position), exp on ScalarE split per head-half
    so it starts right after its own dots matmul, attn@v with an
    augmented ones-column producing softmax row-sums for free.
  * Injection queue: every attention iteration can carry one extra PE
    work item (trailing V tiles, next pairs' K/Q chunks, previous pair's
    O-projection partials), so the PE never drains between phases.
  * Normalize split per query half; O partials accumulated in SBUF on
    gpsimd (DVE is near-saturated during attention); only the last
    pair's partial remains after attention. Output DMA on the scalar
    ring (idle once exp is done).
"""

import numpy as np

import concourse.bass as bass
import concourse.mybir as mybir
import concourse.tile as tile
from concourse import bacc
from concourse.bass_utils import run_bass_kernel_spmd
from concourse.masks import make_identity

P = 128
B = 8
N = 1024  # tokens (queries == keys)
D = 512  # model dim
HEADS = 8
DH = 64
INNER = 512
SCALE = DH**-0.5
EPS = 1e-5
F32 = mybir.dt.float32
F16 = mybir.dt.float16

KO = D // P  # 4 contraction tiles
JT = INNER // P  # 4 output-feature tiles (== head pairs)
TT = N // P  # 8 token tiles
IC = 2  # query chunks of 512
NQC = N // IC  # 512
LAG = 6  # attn@v trails dots/exp by this many (kt, ic) steps


def _build_nc():
    nc = bacc.Bacc(
        "TRN2",
        target_bir_lowering=False,
        debug=False,
        num_devices=B,
    )

    xq_d = nc.declare_dram_parameter("xq", [N, D], F16, isOutput=False)
    xkv_d = nc.declare_dram_parameter("xkv", [N, D], F16, isOutput=False)
    wq_d = nc.declare_dram_parameter("wq", [D, INNER], F16, isOutput=False)
    wk_d = nc.declare_dram_parameter("wk", [D, INNER], F16, isOutput=False)
    wv_d = nc.declare_dram_parameter("wv", [D, INNER], F16, isOutput=False)
    wo_d = nc.declare_dram_parameter("wo", [INNER, D], F16, isOutput=False)
    bq_d = nc.declare_dram_parameter("bq", [INNER], F32, isOutput=False)
    bk_d = nc.declare_dram_parameter("bk", [INNER], F32, isOutput=False)
    bv_d = nc.declare_dram_parameter("bv", [INNER], F32, isOutput=False)
    out_d = nc.declare_dram_parameter("out", [N, D], F32, isOutput=True)

    with tile.TileContext(nc) as tc:
        with (
            tc.tile_pool(name="singles", bufs=1) as singles,
            tc.tile_pool(name="big", bufs=1) as big,
            tc.tile_pool(name="work", bufs=3) as work,
            tc.tile_pool(name="ps", bufs=2, space="PSUM") as ps,
        ):
            # ---- inputs / weights / constants ----
            xin_kv = big.tile([P, TT, D], F16)
            xin_q = big.tile([P, TT, D], F16)
            H = TT // 2
            for h in range(2):
                nc.sync.dma_start(
                    out=xin_kv[:, h * H : (h + 1) * H, :],
                    in_=xkv_d[h * H * P : (h + 1) * H * P, :].rearrange(
                        "(tt p) j -> p tt j", p=P
                    ),
                )
            for h in range(2):
                nc.scalar.dma_start(
                    out=xin_q[:, h * H : (h + 1) * H, :],
                    in_=xq_d[h * H * P : (h + 1) * H * P, :].rearrange(
                        "(tt p) j -> p tt j", p=P
                    ),
                )

            wq_sb = singles.tile([P, KO, INNER], F16)
            wk_sb = singles.tile([P, KO, INNER], F16)
            wv_sb = singles.tile([P, KO, INNER], F16)
            wo_sb = singles.tile([P, KO, D], F16)
            bq_sb = singles.tile([P, JT], F32)
            bk_sb = singles.tile([P, JT], F32)
            bvB = singles.tile([P, INNER], F32)
            ident = singles.tile([P, P], F16)
            make_identity(nc, ident)
            # weight rows laid out to match the xbar transpose ordering:
            # feature j of xhat lands at xhatT[p=j%128, ko=j//128]
            nc.gpsimd.dma_start(out=wv_sb[:], in_=wv_d.rearrange("(ko p) j -> p ko j", p=P))
            nc.gpsimd.dma_start(out=wk_sb[:], in_=wk_d.rearrange("(ko p) j -> p ko j", p=P))
            nc.gpsimd.dma_start(out=bq_sb[:], in_=bq_d.rearrange("(t p) -> p t", p=P))
            nc.gpsimd.dma_start(out=bk_sb[:], in_=bk_d.rearrange("(t p) -> p t", p=P))
            bv_ap = bv_d.ap()
            bv_bcast = bass.AP(tensor=bv_ap.tensor, offset=bv_ap.offset, ap=[[0, P], [1, INNER]])
            nc.gpsimd.dma_start(out=bvB[:], in_=bv_bcast)
            nc.gpsimd.dma_start(out=wq_sb[:], in_=wq_d.rearrange("(ko p) j -> p ko j", p=P))
            nc.gpsimd.dma_start(out=wo_sb[:], in_=wo_d.rearrange("(co p) j -> p co j", p=P))

            eps_sb = singles.tile([P, 1], F32)
            nc.vector.memset(eps_sb, EPS)

            # ---- persistent activations ----
            xhatT_q = big.tile([P, KO, N], F16)  # [d%128, d//128, token]
            xhatT_kv = big.tile([P, KO, N], F16)
            QT = big.tile([P, JT, N], F16)  # [j%128, j//128, token]
            KT = big.tile([P, JT, N], F16)
            Vg = big.tile([P, TT, HEADS, DH + 1], F16)  # [key%128, keytile, h, dh|1]
            outT = big.tile([P, KO, N], F16)  # [c%128, c//128(=pair), token]
            # fp16 accumulator: halves the DVE cost of the adds; the partials
            # are O(0.3) so fp16 rounding is ~1e-3 relative, well in budget
            oacc = big.tile([P, TT, D], F16)  # O-proj partial accumulator

            ones_sb = singles.tile([P, 1], F32)
            nc.vector.memset(ones_sb, 1.0)
            nc.vector.tensor_copy(
                out=Vg[:, :, :, DH : DH + 1],
                in_=ones_sb[:, None, None, :].to_broadcast((P, TT, HEADS, 1)),
            )

            # ---- stage emitters ----
            def ln_head(xt):
                """Shared LN stats chain; returns (rstd, nmr) partition APs."""
                stats = work.tile([P, 6], F32, tag="ln_stats")
                nc.vector.bn_stats(out=stats[:], in_=xt)
                mv = work.tile([P, 2], F32, tag="ln_mv")
                nc.vector.bn_aggr(out=mv[:], in_=stats[:])
                std = work.tile([P, 1], F32, tag="ln_std")
                nc.scalar.activation(
                    out=std[:],
                    in_=mv[:, 1:2],
                    func=mybir.ActivationFunctionType.Sqrt,
                    bias=eps_sb[:],
                    scale=1.0,
                )
                rstd = work.tile([P, 1], F32, tag="ln_rstd")
                nc.vector.reciprocal(out=rstd[:], in_=std[:])
                # nmr = -mu * rstd (per-partition bias for the ScalarE affine)
                nmr = work.tile([P, 1], F32, tag="ln_nmr")
                nc.gpsimd.tensor_scalar(
                    out=nmr[:],
                    in0=mv[:, 0:1],
                    scalar1=rstd[:, 0:1],
                    scalar2=-1.0,
                    op0=mybir.AluOpType.mult,
                    op1=mybir.AluOpType.mult,
                )
                return rstd, nmr

            def ln_tr_kv(tt):
                """kv tile: LN + XBAR transpose on the sync ring."""
                xt = xin_kv[:, tt, :]
                rstd, nmr = ln_head(xt)
                xhat = work.tile([P, D], F16, tag="xhat", bufs=4)
                nc.scalar.activation(
                    out=xhat[:],
                    in_=xt,
                    func=mybir.ActivationFunctionType.Identity,
                    bias=nmr[:],
                    scale=rstd[:],
                )
                nc.sync.dma_start_transpose(
                    out=xhatT_kv[:, :, tt * P : (tt + 1) * P], in_=xhat[:]
                )

            def ln_tr_q(tt):
                """q tile: LN + PE transpose + gpsimd copyback."""
                xt = xin_q[:, tt, :]
                rstd, nmr = ln_head(xt)
                xhat = work.tile([P, D], F16, tag="xhat", bufs=4)
                nc.scalar.activation(
                    out=xhat[:],
                    in_=xt,
                    func=mybir.ActivationFunctionType.Identity,
                    bias=nmr[:],
                    scale=rstd[:],
                )
                pt = ps.tile([P, D], F16, tag="attnv")
                for db in range(KO):
                    nc.tensor.transpose(
                        pt[:, db * P : (db + 1) * P], xhat[:, db * P : (db + 1) * P], ident[:]
                    )
                nc.vector.tensor_copy(
                    out=xhatT_q[:, :, tt * P : (tt + 1) * P],
                    in_=pt[:].rearrange("p (ko t) -> p ko t", t=P),
                )

            def v_proj(tt):
                """V projection (token-major) into the augmented V tile."""
                pm = ps.tile([P, INNER], F32, tag="big")
                for ko in range(KO):
                    nc.tensor.matmul(
                        pm[:],
                        xhatT_kv[:, ko, tt * P : (tt + 1) * P],
                        wv_sb[:, ko, :],
                        start=(ko == 0),
                        stop=(ko == KO - 1),
                    )
                nc.vector.tensor_tensor(
                    out=Vg[:, tt, :, 0:DH],
                    in0=pm[:].rearrange("p (h d) -> p h d", d=DH),
                    in1=bvB.rearrange("p (h d) -> p h d", d=DH),
                    op=mybir.AluOpType.add,
                )

            def kq_proj(w_sb, b_sb, src, dstT, jt, ic):
                """K^T or Q^T projection for feature tile jt, chunk ic."""
                pm = ps.tile([P, NQC], F32, tag="big")
                for ko in range(KO):
                    nc.tensor.matmul(
                        pm[:],
                        w_sb[:, ko, jt * P : (jt + 1) * P],
                        src[:, ko, ic * NQC : (ic + 1) * NQC],
                        start=(ko == 0),
                        stop=(ko == KO - 1),
                    )
                nc.vector.tensor_scalar(
                    out=dstT[:, jt, ic * NQC : (ic + 1) * NQC],
                    in0=pm[:],
                    scalar1=b_sb[:, jt : jt + 1],
                    scalar2=None,
                    op0=mybir.AluOpType.add,
                )

            def k_proj(jt, ic):
                kq_proj(wk_sb, bk_sb, xhatT_kv, KT, jt, ic)

            def q_proj(jt, ic):
                kq_proj(wq_sb, bq_sb, xhatT_q, QT, jt, ic)

            def attention_pair(hq, queue=()):
                """Heads 2*hq and 2*hq+1 together: their dots matmuls use
                disjoint PE row groups (K=64 at base partitions 0 and 64) and
                run concurrently into different PSUM banks. One thunk from
                `queue` is emitted per (kt, ic) iteration so the PE always has
                independent fill work. exp is split per head-half so each half
                starts right after its own dots matmul."""
                h0, h1 = 2 * hq, 2 * hq + 1
                po0 = ps.tile([DH + 1, N], F32, tag="attnv", name="po0")
                po1 = ps.tile([DH + 1, N], F32, tag="attnv", name="po1")
                queue = list(queue)
                exs = []
                for kt in range(TT):
                    for ic in range(IC):
                        # pd holds head-even in the first bank, head-odd in
                        # the second; the two matmuls run concurrently
                        pd = ps.tile([P, N], F32, tag="big", name="pd")
                        ex = work.tile([P, N], F16, tag="expT", bufs=LAG + 2)
                        for hh in range(2):
                            nc.tensor.matmul(
                                pd[:, hh * NQC : (hh + 1) * NQC],
                                KT[hh * DH : (hh + 1) * DH, hq, kt * P : (kt + 1) * P],
                                QT[hh * DH : (hh + 1) * DH, hq, ic * NQC : (ic + 1) * NQC],
                                start=True,
                                stop=True,
                                tile_position=(hh * DH, 0),
                            )
                            nc.scalar.activation(
                                out=ex[:, hh * NQC : (hh + 1) * NQC],
                                in_=pd[:, hh * NQC : (hh + 1) * NQC],
                                func=mybir.ActivationFunctionType.Exp,
                                scale=SCALE,
                            )
                        exs.append(ex)
                        i = len(exs) - 1
                        if queue:
                            thunk = queue.pop(0)
                            if thunk is not None:
                                thunk()
                        if i >= LAG:
                            _attnv_pair(po0, po1, h0, h1, i - LAG, exs[i - LAG])
                for thunk in queue:
                    if thunk is not None:
                        thunk()
                for i in range(TT * IC - LAG, TT * IC):
                    _attnv_pair(po0, po1, h0, h1, i, exs[i])
                # normalize per query half so dependents can start after the
                # (kt=7, ic) flush instead of the full pair
                for ic in range(IC):
                    _normalize(po0, h0, ic)
                    _normalize(po1, h1, ic)

            def _attnv_pair(po0, po1, h0, h1, i, ex):
                # ex holds [head0 chunk ic | head1 chunk ic] for key tile kt
                kt, ic = divmod(i, IC)
                for po, h, hh in ((po0, h0, 0), (po1, h1, 1)):
                    nc.tensor.matmul(
                        po[:, ic * NQC : (ic + 1) * NQC],
                        Vg[:, kt, h, :],
                        ex[:, hh * NQC : (hh + 1) * NQC],
                        start=(kt == 0),
                        stop=(kt == TT - 1),
                    )

            def _normalize(po, h, ic):
                # out^T = po[0:64] * (1/rowsum) broadcast over partitions
                hb = (h % 2) * DH
                hq = h // 2
                sl = slice(ic * NQC, (ic + 1) * NQC)
                rtmp = work.tile([1, 2 * NQC], F32, tag="rectmp")
                rs, rec = rtmp[:, 0:NQC], rtmp[:, NQC : 2 * NQC]
                nc.vector.tensor_copy(out=rs, in_=po[DH : DH + 1, sl])
                nc.vector.reciprocal_approx_fast(out=rec, in_=rs)
                recB = work.tile([DH, NQC], F32, tag="recB")
                nc.gpsimd.partition_broadcast(recB[:], rec[:])
                nc.vector.tensor_tensor(
                    out=outT[hb : hb + DH, hq, sl],
                    in0=po[0:DH, sl],
                    in1=recB[:],
                    op=mybir.AluOpType.mult,
                )

            def o_part(q, tt):
                """O-projection contribution of head pair q (contraction tile
                co == q) for token tile tt, accumulated into oacc on gpsimd;
                the last pair finishes and DMAs out on the scalar ring."""
                pm = ps.tile([P, D], F32, tag="big")
                nc.tensor.matmul(
                    pm[:],
                    outT[:, q, tt * P : (tt + 1) * P],
                    wo_sb[:, q, :],
                    start=True,
                    stop=True,
                )
                if q == 0:
                    nc.vector.tensor_copy(out=oacc[:, tt, :], in_=pm[:])
                elif q < JT - 1:
                    nc.vector.tensor_tensor(
                        out=oacc[:, tt, :], in0=oacc[:, tt, :], in1=pm[:],
                        op=mybir.AluOpType.add,
                    )
                else:
                    ot = work.tile([P, D], F32, tag="out")
                    nc.vector.tensor_tensor(
                        out=ot[:], in0=oacc[:, tt, :], in1=pm[:],
                        op=mybir.AluOpType.add,
                    )
                    nc.scalar.dma_start(out=out_d[tt * P : (tt + 1) * P, :], in_=ot[:])

            # ---- emission: LN interleaved kv/q 4-and-4 (matches the two
            # half-tensor DMA arrivals per ring), minimal pre-attention
            # projections, everything else on the injection queues ----
            for tt in range(4):
                ln_tr_kv(tt)
            for tt in range(4):
                ln_tr_q(tt)
            for tt in range(4, TT):
                ln_tr_kv(tt)
            for tt in range(4, TT):
                ln_tr_q(tt)
            v_proj(0)
            k_proj(0, 0)
            k_proj(0, 1)
            q_proj(0, 0)
            q_proj(0, 1)

            def kq(jt):
                return [
                    lambda: k_proj(jt, 0),
                    lambda: k_proj(jt, 1),
                    lambda: q_proj(jt, 0),
                    lambda: q_proj(jt, 1),
                ]

            def oparts(q):
                return [lambda tt=tt: o_part(q, tt) for tt in range(TT)]

            attention_pair(0, queue=[lambda tt=tt: v_proj(tt) for tt in range(1, TT)] + kq(1))
            attention_pair(1, queue=kq(2) + oparts(0))
            attention_pair(2, queue=kq(3) + oparts(1))
            attention_pair(3, queue=[None] * 4 + oparts(2))
            for tt in range(TT):
                o_part(3, tt)

    nc.compile()
    return nc


_NC_CACHE = {}


def _get_nc():
    if "nc" not in _NC_CACHE:
        _NC_CACHE["nc"] = _build_nc()
    return _NC_CACHE["nc"]


def _prep_in_maps(query, keyvalue, Wq, Wkv, Wo, gamma, beta):
    query = np.asarray(query, dtype=np.float32)
    keyvalue = np.asarray(keyvalue, dtype=np.float32)
    Wq = np.asarray(Wq, dtype=np.float32)
    Wkv = np.asarray(Wkv, dtype=np.float32)
    Wo = np.asarray(Wo, dtype=np.float32)
    gamma = np.asarray(gamma, dtype=np.float32)
    beta = np.asarray(beta, dtype=np.float32)

    # fold LN affine into the projections: (xhat*g + b) @ W = xhat @ (g[:,None]*W) + b @ W
    wq_eff = np.ascontiguousarray((gamma[:, None] * Wq).astype(np.float16))
    wkv_eff = gamma[:, None] * Wkv
    bq = np.ascontiguousarray(beta @ Wq)
    bkv = beta @ Wkv
    wk_eff = np.ascontiguousarray(wkv_eff[:, :INNER].astype(np.float16))
    wv_eff = np.ascontiguousarray(wkv_eff[:, INNER:].astype(np.float16))
    bk = np.ascontiguousarray(bkv[:INNER])
    bv = np.ascontiguousarray(bkv[INNER:])
    wo_eff = np.ascontiguousarray(Wo.astype(np.float16))

    return [
        dict(
            xq=np.ascontiguousarray(query[b].astype(np.float16)),
            xkv=np.ascontiguousarray(keyvalue[b].astype(np.float16)),
            wq=wq_eff,
            wk=wk_eff,
            wv=wv_eff,
            wo=wo_eff,
            bq=bq,
            bk=bk,
            bv=bv,
        )
        for b in range(B)
    ]


def run_sharded(inputs, **spmd_kwargs):
    """Run the SPMD kernel; returns (stacked output [B, N, D], BassKernelResults)."""
    nc = _get_nc()
    in_maps = _prep_in_maps(**inputs)
    r = run_bass_kernel_spmd(nc, in_maps, core_ids=list(range(B)), **spmd_kwargs)
    out = np.stack([r.results[b]["out"] for b in range(B)], axis=0)
    return out, r


def kernel(query, keyvalue, Wq, Wkv, Wo, gamma, beta):
    out, _ = run_sharded(
        dict(query=query, keyvalue=keyvalue, Wq=Wq, Wkv=Wkv, Wo=Wo, gamma=gamma, beta=beta)
    )
    return out
